# revision 15
# baseline (speedup 1.0000x reference)
"""Trainium2 Bass kernel for nn_Cross_head (sparse_attention patch-correction).

Math (non-overlapping unfold/fold are inverse permutations, so
corr = y + fold(attentions/nz)):
    y   = W @ x + b                       (1x1x1 conv over channels)
    out = leaky_relu(y * (y + foldA + 1), 0.2)
    foldA = fold(attentions / (count_nonzero(attentions, -1) + 1e-5))

Sharding: spatial, across the 576 patch-columns (72 per core); no
cross-core communication.

DMA-bound problem, so inputs are staged bf16 (host casts) and the
output is stored bf16 and upcast on the host (tolerance 2e-2; bf16
end-to-end lands ~5e-3).  DRAM layouts are host-permuted so every DMA
run is one contiguous 5832B row per channel per subtile.  All DMA is
HWDGE: loads on the sync queue, stores on the scalar queue.

att arrives already in voxel order (p1, iw, p2), so every big DVE op
is contiguous bf16->bf16 (fast mode).  The nonzero count runs as two
reduces: over p2 (contiguous runs of 9) then over p1 (strided).  The
psum->SBUF conversion u = y0 + b rides on identity-Prelu activations
(alpha=1) split between the scalar engine and psum-capable peers.

Per subtile (9 d-rows x 36 patch columns, 2916 voxels/channel):
  sync   : x load, att load (HWDGE)
  tensor : 6 bf16 matmuls into 2 psum groups
  vector : st=(att!=0); R1=sum_p2; R2=sum_p1; 1/nz; A=att*r (bf16);
           t=(A+1)+u; pre=u*t
  scalar : u = Prelu_id(psum+b) (one group); lrelu=Prelu; out store
  gpsimd : u of the other psum group; A-mult alternates here
"""

import os
import sys

import numpy as np

sys.path.insert(0, "/opt/trn_rl_repo")

# ---- geometry (hardcoded for this problem) ----
C = 128          # channels (in == out)
D = 36           # depth
HWFULL = 5184    # H*W = 72*72
PS = 9           # patch size
NDP = 4          # D // PS
NWP = 576        # HWFULL // PS  (patch columns)
NCORES = 8
IWG = NWP // NCORES   # 72 patch columns per core
HWL = IWG * PS        # 648 voxel columns per core
NSUB = 2              # split each iD block into halves along iW
IWT = IWG // NSUB     # 36 patch columns per subtile
FT = IWT * 81         # 2916 free elements per subtile
HWT = IWT * PS        # 324 voxel columns per subtile
MMN = 486             # matmul free dim: contiguous voxel slice
NMM = PS * HWT // MMN # 6 matmuls per subtile
NGRP = 2              # psum groups per subtile
MMG = NMM // NGRP     # 3 matmuls per psum group
BANK = 512            # fp32 elements per PSUM bank
NT = NDP * NSUB       # 8 subtiles per core

_NC_CACHE = {}
LAST_RESULT = None


def _build_nc(att_dtype="bfloat16", lrelu_engine="scalar", amul_engine="gpsimd",
              u_engine2="vector", pre_engine="gpsimd"):
    from contextlib import ExitStack

    import concourse.bacc as bacc
    import concourse.tile as tile
    from concourse import mybir

    f32 = mybir.dt.float32
    bf16 = mybir.dt.bfloat16
    AL = mybir.AluOpType
    AF = mybir.ActivationFunctionType
    adt = getattr(mybir.dt, att_dtype)

    nc = bacc.Bacc(
        "TRN2",
        target_bir_lowering=False,
        debug=False,
        enable_asserts=False,
        num_devices=NCORES,
    )
    x_d = nc.dram_tensor("x", [C, NT, PS * HWT], bf16, kind="ExternalInput").ap()
    a_d = nc.dram_tensor("att", [C, NT, FT], adt, kind="ExternalInput").ap()
    wt_d = nc.dram_tensor("wt", [C, C], bf16, kind="ExternalInput").ap()
    b_d = nc.dram_tensor("bias", [C, 2], f32, kind="ExternalInput").ap()
    o_d = nc.dram_tensor("out", [C, NT, PS * HWT], bf16, kind="ExternalOutput").ap()

    with tile.TileContext(nc) as tc, ExitStack() as ctx:
        const = ctx.enter_context(tc.tile_pool(name="const", bufs=1))
        wt_sb = const.tile([C, C], bf16)
        nc.sync.dma_start(wt_sb[:], wt_d[:])
        b_sb = const.tile([C, 2], f32)
        nc.sync.dma_start(b_sb[:], b_d[:])
        b_ap = b_sb[:, 0:1]
        alpha_sb = const.tile([C, 2], f32)
        nc.vector.memset(alpha_sb[:, 0:1], 0.2)
        nc.vector.memset(alpha_sb[:, 1:2], 1.0)

        xp = ctx.enter_context(tc.tile_pool(name="xp", bufs=3))
        atp = ctx.enter_context(tc.tile_pool(name="atp", bufs=3))
        sgp = ctx.enter_context(tc.tile_pool(name="sgp", bufs=2))
        r1p = ctx.enter_context(tc.tile_pool(name="r1p", bufs=2))
        nzp = ctx.enter_context(tc.tile_pool(name="nzp", bufs=2))
        Apl = ctx.enter_context(tc.tile_pool(name="Apl", bufs=2))
        upl = ctx.enter_context(tc.tile_pool(name="upl", bufs=2))
        tpl = ctx.enter_context(tc.tile_pool(name="tpl", bufs=2))
        prp = ctx.enter_context(tc.tile_pool(name="prp", bufs=2))
        ovp = ctx.enter_context(tc.tile_pool(name="ovp", bufs=3))
        psp = ctx.enter_context(tc.tile_pool(name="psp", bufs=2, space="PSUM"))

        def issue_loads(sub):
            xt = xp.tile([C, PS * HWT], bf16, name=f"xt{sub}", tag="xt")
            nc.sync.dma_start(xt[:], x_d[:, sub, :])
            at = atp.tile([C, FT], adt, name=f"at{sub}", tag="at")
            nc.sync.dma_start(at[:], a_d[:, sub, :])
            return xt, at

        loaded = {0: issue_loads(0), 1: issue_loads(1)}

        for sub in range(NT):
            xt, at = loaded.pop(sub)
            if sub + 2 < NT:
                loaded[sub + 2] = issue_loads(sub + 2)

            # ---- nz = count_nonzero per patch, two-step reduce ----
            st = sgp.tile([C, FT], bf16)
            nc.vector.tensor_scalar(st[:], at[:], 0.0, None, AL.not_equal)
            r1 = r1p.tile([C, PS * IWT], bf16)  # (p1, iw) counts over p2
            with nc.allow_low_precision(reason="counts <= 9 exact in bf16"):
                nc.vector.tensor_reduce(
                    r1[:],
                    st[:].rearrange("c (pw q) -> c pw q", q=PS),
                    mybir.AxisListType.X,
                    AL.add,
                )
            nzv = nzp.tile([C, IWT], f32)
            nc.vector.tensor_reduce(
                nzv[:],
                r1[:].rearrange("c (p w) -> c w p", p=PS),
                mybir.AxisListType.X,
                AL.add,
            )
            nzrf = nzp.tile([C, IWT], f32)
            nzr = nzp.tile([C, IWT], bf16)
            nc.vector.tensor_scalar_add(nzv[:], nzv[:], 1e-5)
            nc.vector.reciprocal_approx_fast(nzrf[:], nzv[:])
            nc.vector.tensor_scalar_add(nzr[:], nzrf[:], 0.0)

            # ---- A = att * (1/nz); att already voxel order, contiguous --
            At = Apl.tile([C, FT], bf16)
            a3 = at[:].rearrange("c (p w q) -> c p w q", p=PS, q=PS)
            nzr3 = (
                nzr[:]
                .unsqueeze(1)
                .unsqueeze(3)
                .broadcast_to((C, PS, IWT, PS))
            )
            A3 = At[:].rearrange("c (p w q) -> c p w q", p=PS, q=PS)
            if amul_engine == "split":
                amul = nc.vector if sub % 2 == 0 else nc.gpsimd
            else:
                amul = getattr(nc, amul_engine)
            amul.tensor_tensor(A3, a3, nzr3, AL.mult)

            # ---- GEMM: psum = W @ x, plain contiguous voxel slices ----
            pst = []
            for g in range(NGRP):
                ps_t = psp.tile([C, MMG * BANK], f32)  # 3 banks
                pst.append(ps_t)
                for m in range(MMG):
                    ch = g * MMG + m
                    nc.tensor.matmul(
                        ps_t[:, m * BANK : m * BANK + MMN],
                        wt_sb[:],
                        xt[:, ch * MMN : (ch + 1) * MMN],
                        start=True,
                        stop=True,
                    )

            # ---- u = y0 + b  (identity-Prelu, psum -> bf16 SBUF) ----
            ut = upl.tile([C, FT], bf16)
            for g in range(NGRP):
                ps_ap = (
                    pst[g][:]
                    .rearrange("c (m n) -> c m n", n=BANK)[:, :, 0:MMN]
                )  # [C, 3, 486]
                sl = slice(g * MMG * MMN, (g + 1) * MMG * MMN)
                u2 = ut[:, sl].rearrange("c (m n) -> c m n", n=MMN)
                if g == 0 or u_engine2 == "scalar":
                    nc.scalar.activation(
                        u2, ps_ap, AF.Prelu, bias=b_ap, alpha=alpha_sb[:, 1:2]
                    )
                else:
                    # psum + b on DVE (gpsimd cannot touch PSUM)
                    nc.vector.tensor_scalar(u2, ps_ap, b_ap, None, AL.add)

            # ---- t = (A + 1) + u ; pre = u * t   (bf16, contiguous) ----
            tt = tpl.tile([C, FT], bf16)
            nc.vector.scalar_tensor_tensor(
                tt[:], At[:], 1.0, ut[:], AL.add, AL.add
            )
            pre = prp.tile([C, FT], bf16)
            peng = getattr(nc, pre_engine)
            peng.tensor_tensor(pre[:], ut[:], tt[:], AL.mult)

            # ---- out = lrelu(pre) ----
            ov = ovp.tile([C, PS * HWT], bf16)
            if lrelu_engine == "split":
                leng = "scalar" if sub % 2 == 0 else "vector"
            else:
                leng = lrelu_engine
            if leng == "scalar":
                nc.scalar.activation(
                    ov[:], pre[:], AF.Prelu, alpha=alpha_sb[:, 0:1]
                )
            else:
                nc.vector.scalar_tensor_tensor(
                    ov[:], pre[:], 0.2, pre[:], AL.mult, AL.max
                )

            # ---- contiguous store on the scalar HWDGE queue ----
            nc.scalar.dma_start(o_d[:, sub, :], ov[:])

    nc.compile()
    return nc


def _get_nc(**kw):
    key = tuple(sorted(kw.items()))
    if key not in _NC_CACHE:
        _NC_CACHE[key] = _build_nc(**kw)
    return _NC_CACHE[key]


def kernel(x, attentions, W, b, **build_kw):
    global LAST_RESULT
    import ml_dtypes
    from concourse.bass_utils import run_bass_kernel_spmd

    bf16 = ml_dtypes.bfloat16
    att_np = {"bfloat16": bf16, "float8e4": ml_dtypes.float8_e4m3}[
        build_kw.get("att_dtype", "bfloat16")
    ]

    x = np.asarray(x, dtype=np.float32)
    attentions = np.asarray(attentions, dtype=np.float32)
    W = np.asarray(W, dtype=np.float32)
    b = np.asarray(b, dtype=np.float32)

    nc = _get_nc(**build_kw)

    # x: [C, NDP, PS, NCORES, NSUB, HWT] -> core-major, subtile-contiguous
    xs = (
        x.reshape(C, NDP, PS, NCORES, NSUB, HWT)
        .transpose(3, 0, 1, 4, 2, 5)
        .reshape(NCORES, C, NT, PS * HWT)
        .astype(bf16)
    )
    # att -> voxel order (p1, iw, p2) per subtile, core-major
    as_ = (
        attentions.reshape(C, NDP, NCORES, NSUB, IWT, PS, PS)
        .transpose(2, 0, 1, 3, 5, 4, 6)
        .reshape(NCORES, C, NT, FT)
        .astype(att_np)
    )
    wt = np.ascontiguousarray(W.T).astype(bf16)
    bcol = np.ascontiguousarray(np.stack([b, b + 1.0], axis=1))

    in_maps = []
    for s in range(NCORES):
        in_maps.append(
            {
                "x": np.ascontiguousarray(xs[s]),
                "att": np.ascontiguousarray(as_[s]),
                "wt": wt,
                "bias": bcol,
            }
        )

    res = run_bass_kernel_spmd(
        nc,
        in_maps,
        core_ids=list(range(NCORES)),
        trace=bool(os.environ.get("BASS_TRACE")),
    )
    LAST_RESULT = res

    out = np.empty((NCORES, C, NT, PS * HWT), dtype=np.float32)
    for s in range(NCORES):
        out[s] = res.results[s]["out"].astype(np.float32)
    # inverse of the x permutation
    full = (
        out.reshape(NCORES, C, NDP, NSUB, PS, HWT)
        .transpose(1, 2, 4, 0, 3, 5)
        .reshape(1, C, D, HWFULL)
    )
    return full


# revision 16
# speedup vs baseline: 1.7294x; 1.7294x over previous
"""Trainium2 Bass kernel for nn_Cross_head (sparse_attention patch-correction).

Math (non-overlapping unfold/fold are inverse permutations, so
corr = y + fold(attentions/nz)):
    y   = W @ x + b                       (1x1x1 conv over channels)
    out = leaky_relu(y * (y + foldA + 1), 0.2)
    foldA = fold(attentions / (count_nonzero(attentions, -1) + 1e-5))

Sharding: spatial, across the 576 patch-columns (72 per core); no
cross-core communication.

DMA-bound problem: inputs staged bf16 (host casts), output stored bf16
and upcast on host (tolerance 2e-2, this lands ~4e-3).  DRAM layouts
host-permuted so every DMA run is one contiguous 5832B row per channel
per subtile.  All DMA is HWDGE: loads on sync, stores on scalar queue.

The whole elementwise tail is ONE custom DVE op (registered in-process,
the documented extension path for ant custom-DVE ops):
    Z   = (psum + b) * (psum + A + (b+1))     # = y*(y + A + 1)
    out = max(Z, 0.2*Z)                        # LeakyReLU(0.2)
computed in the DVE fp32 pipeline straight from PSUM, written as bf16.

Per subtile (9 d-rows x 36 patch columns, 2916 voxels/channel):
  sync   : x load, att load (HWDGE)
  scalar : st = Sign(att) for the nonzero count; out store (HWDGE)
  vector : R1 = sum_p2(|st|); R2 = sum_p1; clamp; 1/nz; tail op x2
  gpsimd : A = att * (1/nz) with 4D broadcast of 1/nz
  tensor : 6 bf16 matmuls into 2 psum groups
"""

import os
import sys

import numpy as np

sys.path.insert(0, "/opt/trn_rl_repo")

# ---- geometry (hardcoded for this problem) ----
C = 128          # channels (in == out)
D = 36           # depth
HWFULL = 5184    # H*W = 72*72
PS = 9           # patch size
NDP = 4          # D // PS
NWP = 576        # HWFULL // PS  (patch columns)
NCORES = 8
IWG = NWP // NCORES   # 72 patch columns per core
HWL = IWG * PS        # 648 voxel columns per core
NSUB = 2              # split each iD block into halves along iW
IWT = IWG // NSUB     # 36 patch columns per subtile
FT = IWT * 81         # 2916 free elements per subtile
HWT = IWT * PS        # 324 voxel columns per subtile
MMN = 486             # matmul free dim: contiguous voxel slice
NMM = PS * HWT // MMN # 6 matmuls per subtile
NGRP = 2              # psum groups per subtile
MMG = NMM // NGRP     # 3 matmuls per psum group
BANK = 512            # fp32 elements per PSUM bank
NT = NDP * NSUB       # 8 subtiles per core

_NC_CACHE = {}
_TAIL_OP = None
LAST_RESULT = None


def _tail_ref(in0, in1, s0, s1, imm2):
    z = (in0.astype(np.float32) + s0) * (in0.astype(np.float32) + in1 + s1)
    return np.maximum(z, z * imm2).astype(np.float32)


def _get_tail_op():
    """Register the fused tail op with the ant custom-DVE table:
    out = max(Z, imm2*Z), Z = (in0 + s0)*(in0 + in1 + s1)."""
    global _TAIL_OP
    if _TAIL_OP is not None:
        return _TAIL_OP
    from concourse import dve_ops
    from concourse.dve_spec import C0, C1, C2, Spec, Src0, Src1, lower, maxx
    from concourse.dve_uop import DveOpSpec

    name = "CROSS_HEAD_TAIL_ANT"
    for op in dve_ops.OPS:
        if op.name == name:
            _TAIL_OP = op
            return op

    Z = (Src0 + C0) * (Src0 + Src1 + C1)
    spec = Spec(body=maxx(Z, Z * C2), reference=_tail_ref)
    row = max(dve_ops._SUB_OPCODE_FOR_NAME.values()) + 1
    assert row < 0x20
    shas = {}
    for ver in ("v3", "v4"):
        try:
            uops = lower(spec, ver=ver)
        except Exception:
            continue
        shas[ver] = DveOpSpec(
            name=name, opcode=row, uops=uops, rd1_en=True
        ).sha(ver)
    op = dve_ops.DveOp(name, spec, subdim=False, uops_sha=shas)
    dve_ops.OPS.append(op)
    dve_ops._SUB_OPCODE_FOR_NAME[name] = row
    dve_ops.CUSTOM_DVE_SPECS[name] = spec
    _TAIL_OP = op
    return op


def _build_nc(att_dtype="bfloat16", amul_engine="gpsimd", sign_engine="scalar"):
    from contextlib import ExitStack

    import concourse.bacc as bacc
    import concourse.tile as tile
    from concourse import mybir

    tail_op = _get_tail_op()

    f32 = mybir.dt.float32
    bf16 = mybir.dt.bfloat16
    AL = mybir.AluOpType
    AF = mybir.ActivationFunctionType
    adt = getattr(mybir.dt, att_dtype)

    nc = bacc.Bacc(
        "TRN2",
        target_bir_lowering=False,
        debug=False,
        enable_asserts=False,
        num_devices=NCORES,
    )
    x_d = nc.dram_tensor("x", [C, NT, PS * HWT], bf16, kind="ExternalInput").ap()
    a_d = nc.dram_tensor("att", [C, NT, FT], adt, kind="ExternalInput").ap()
    wt_d = nc.dram_tensor("wt", [C, C], bf16, kind="ExternalInput").ap()
    b_d = nc.dram_tensor("bias", [C, 2], f32, kind="ExternalInput").ap()
    o_d = nc.dram_tensor("out", [C, NT, PS * HWT], bf16, kind="ExternalOutput").ap()

    with tile.TileContext(nc) as tc, ExitStack() as ctx:
        const = ctx.enter_context(tc.tile_pool(name="const", bufs=1))
        wt_sb = const.tile([C, C], bf16)
        nc.sync.dma_start(wt_sb[:], wt_d[:])
        b_sb = const.tile([C, 2], f32)
        nc.sync.dma_start(b_sb[:], b_d[:])
        b_ap = b_sb[:, 0:1]
        bp1_ap = b_sb[:, 1:2]

        xp = ctx.enter_context(tc.tile_pool(name="xp", bufs=3))
        atp = ctx.enter_context(tc.tile_pool(name="atp", bufs=3))
        sgp = ctx.enter_context(tc.tile_pool(name="sgp", bufs=2))
        r1p = ctx.enter_context(tc.tile_pool(name="r1p", bufs=2))
        nzp = ctx.enter_context(tc.tile_pool(name="nzp", bufs=2))
        Apl = ctx.enter_context(tc.tile_pool(name="Apl", bufs=2))
        ovp = ctx.enter_context(tc.tile_pool(name="ovp", bufs=3))
        psp = ctx.enter_context(tc.tile_pool(name="psp", bufs=2, space="PSUM"))

        def issue_loads(sub):
            xt = xp.tile([C, PS * HWT], bf16, name=f"xt{sub}", tag="xt")
            nc.sync.dma_start(xt[:], x_d[:, sub, :])
            at = atp.tile([C, FT], adt, name=f"at{sub}", tag="at")
            nc.sync.dma_start(at[:], a_d[:, sub, :])
            return xt, at

        loaded = {0: issue_loads(0), 1: issue_loads(1)}

        for sub in range(NT):
            xt, at = loaded.pop(sub)
            if sub + 2 < NT:
                loaded[sub + 2] = issue_loads(sub + 2)

            # ---- nz = count_nonzero per patch ----
            st = sgp.tile([C, FT], bf16)
            if sign_engine == "scalar":
                nc.scalar.activation(st[:], at[:], AF.Sign)
            else:
                nc.vector.tensor_scalar(st[:], at[:], 0.0, None, AL.not_equal)
            r1 = r1p.tile([C, PS * IWT], bf16)  # (p1, iw) counts over p2
            with nc.allow_low_precision(reason="counts <= 9 exact in bf16"):
                nc.vector.tensor_reduce(
                    r1[:],
                    st[:].rearrange("c (pw q) -> c pw q", q=PS),
                    mybir.AxisListType.X,
                    AL.add,
                    apply_absolute_value=True,
                )
            nzv = nzp.tile([C, IWT], f32)
            nc.vector.tensor_reduce(
                nzv[:],
                r1[:].rearrange("c (p w) -> c w p", p=PS),
                mybir.AxisListType.X,
                AL.add,
            )
            # clamp to >=0.5 instead of +1e-5: identical output (for nz>=1
            # the 1e-5 shift is ~1e-7 relative; for nz=0 every att in the
            # patch is 0 so A = 0 either way), and it keeps 1/nz finite.
            nzc = nzp.tile([C, IWT], f32)
            nc.vector.tensor_scalar_max(nzc[:], nzv[:], 0.5)
            nzr = nzp.tile([C, IWT], f32)
            nc.vector.reciprocal_approx_fast(nzr[:], nzc[:])

            # ---- A = att * (1/nz); att already voxel order, contiguous --
            At = Apl.tile([C, FT], bf16)
            a3 = at[:].rearrange("c (p w q) -> c p w q", p=PS, q=PS)
            nzr3 = (
                nzr[:]
                .unsqueeze(1)
                .unsqueeze(3)
                .broadcast_to((C, PS, IWT, PS))
            )
            A3 = At[:].rearrange("c (p w q) -> c p w q", p=PS, q=PS)
            if amul_engine == "split":
                amul = nc.gpsimd if sub % 2 == 0 else nc.vector
            else:
                amul = getattr(nc, amul_engine)
            amul.tensor_tensor(A3, a3, nzr3, AL.mult)

            # ---- GEMM: psum = W @ x, plain contiguous voxel slices ----
            pst = []
            for g in range(NGRP):
                ps_t = psp.tile([C, MMG * BANK], f32)  # 3 banks
                pst.append(ps_t)
                for m in range(MMG):
                    ch = g * MMG + m
                    nc.tensor.matmul(
                        ps_t[:, m * BANK : m * BANK + MMN],
                        wt_sb[:],
                        xt[:, ch * MMN : (ch + 1) * MMN],
                        start=True,
                        stop=True,
                    )

            # ---- fused tail: out = lrelu((y0+b)*(y0+A+b+1), 0.2) ----
            ov = ovp.tile([C, PS * HWT], bf16)
            for g in range(NGRP):
                ps3 = (
                    pst[g][:]
                    .rearrange("c (m n) -> c m n", n=BANK)[:, :, 0:MMN]
                )  # [C, 3, 486]
                sl = slice(g * MMG * MMN, (g + 1) * MMG * MMN)
                nc.vector._custom_dve(
                    tail_op,
                    out=ov[:, sl],
                    in0=ps3,
                    in1=At[:, sl],
                    s0=b_ap,
                    s1=bp1_ap,
                    imm2=0.2,
                )

            # ---- contiguous store on the scalar HWDGE queue ----
            nc.scalar.dma_start(o_d[:, sub, :], ov[:])

    nc.compile()
    return nc


def _get_nc(**kw):
    key = tuple(sorted(kw.items()))
    if key not in _NC_CACHE:
        _NC_CACHE[key] = _build_nc(**kw)
    return _NC_CACHE[key]


def kernel(x, attentions, W, b, **build_kw):
    global LAST_RESULT
    import ml_dtypes
    from concourse.bass_utils import run_bass_kernel_spmd

    bf16 = ml_dtypes.bfloat16
    att_np = {"bfloat16": bf16, "float8e4": ml_dtypes.float8_e4m3}[
        build_kw.get("att_dtype", "bfloat16")
    ]

    x = np.asarray(x, dtype=np.float32)
    attentions = np.asarray(attentions, dtype=np.float32)
    W = np.asarray(W, dtype=np.float32)
    b = np.asarray(b, dtype=np.float32)

    nc = _get_nc(**build_kw)

    # x: [C, NDP, PS, NCORES, NSUB, HWT] -> core-major, subtile-contiguous
    xs = (
        x.reshape(C, NDP, PS, NCORES, NSUB, HWT)
        .transpose(3, 0, 1, 4, 2, 5)
        .reshape(NCORES, C, NT, PS * HWT)
        .astype(bf16)
    )
    # att -> voxel order (p1, iw, p2) per subtile, core-major
    as_ = (
        attentions.reshape(C, NDP, NCORES, NSUB, IWT, PS, PS)
        .transpose(2, 0, 1, 3, 5, 4, 6)
        .reshape(NCORES, C, NT, FT)
        .astype(att_np)
    )
    wt = np.ascontiguousarray(W.T).astype(bf16)
    bcol = np.ascontiguousarray(np.stack([b, b + 1.0], axis=1))

    in_maps = []
    for s in range(NCORES):
        in_maps.append(
            {
                "x": np.ascontiguousarray(xs[s]),
                "att": np.ascontiguousarray(as_[s]),
                "wt": wt,
                "bias": bcol,
            }
        )

    res = run_bass_kernel_spmd(
        nc,
        in_maps,
        core_ids=list(range(NCORES)),
        trace=bool(os.environ.get("BASS_TRACE")),
    )
    LAST_RESULT = res

    out = np.empty((NCORES, C, NT, PS * HWT), dtype=np.float32)
    for s in range(NCORES):
        out[s] = res.results[s]["out"].astype(np.float32)
    # inverse of the x permutation
    full = (
        out.reshape(NCORES, C, NDP, NSUB, PS, HWT)
        .transpose(1, 2, 4, 0, 3, 5)
        .reshape(1, C, D, HWFULL)
    )
    return full


# revision 23
# speedup vs baseline: 1.7686x; 1.0226x over previous
"""Trainium2 Bass kernel for nn_Cross_head (sparse_attention patch-correction).

Math (non-overlapping unfold/fold are inverse permutations, so
corr = y + fold(attentions/nz)):
    y   = W @ x + b                       (1x1x1 conv over channels)
    out = leaky_relu(y * (y + foldA + 1), 0.2)
    foldA = fold(attentions / (count_nonzero(attentions, -1) + 1e-5))

Sharding: spatial, across the 576 patch-columns (72 per core); no
cross-core communication.

DMA-bound problem: inputs staged bf16 (host casts), output stored bf16
and upcast on host (tolerance 2e-2, this lands ~4e-3).  DRAM layouts
host-permuted so every DMA run is one contiguous 5832B row per channel
per subtile.  All DMA is HWDGE: loads on sync, stores on scalar queue.

The whole elementwise tail is ONE custom DVE op (registered in-process,
the documented extension path for ant custom-DVE ops):
    Z   = (psum + b) * (psum + A + (b+1))     # = y*(y + A + 1)
    out = max(Z, 0.2*Z)                        # LeakyReLU(0.2)
computed in the DVE fp32 pipeline straight from PSUM, written as bf16.

Per subtile (9 d-rows x 36 patch columns, 2916 voxels/channel):
  sync   : x load, att load (HWDGE)
  scalar : st = Sign(att) for the nonzero count; out store (HWDGE)
  vector : R1 = sum_p2(|st|); R2 = sum_p1; clamp; 1/nz; tail op x2
  gpsimd : A = att * (1/nz) with 4D broadcast of 1/nz
  tensor : 6 bf16 matmuls into 2 psum groups
"""

import os
import sys

import numpy as np

sys.path.insert(0, "/opt/trn_rl_repo")

# ---- geometry (hardcoded for this problem) ----
C = 128          # channels (in == out)
D = 36           # depth
HWFULL = 5184    # H*W = 72*72
PS = 9           # patch size
NDP = 4          # D // PS
NWP = 576        # HWFULL // PS  (patch columns)
NCORES = 8
IWG = NWP // NCORES   # 72 patch columns per core
HWL = IWG * PS        # 648 voxel columns per core
NSUB = 2              # split each iD block into halves along iW
IWT = IWG // NSUB     # 36 patch columns per subtile
FT = IWT * 81         # 2916 free elements per subtile
HWT = IWT * PS        # 324 voxel columns per subtile
MMN = 486             # matmul free dim: contiguous voxel slice
NMM = PS * HWT // MMN # 6 matmuls per subtile
NGRP = 2              # psum groups per subtile
MMG = NMM // NGRP     # 3 matmuls per psum group
BANK = 512            # fp32 elements per PSUM bank
NT = NDP * NSUB       # 8 subtiles per core

_NC_CACHE = {}
_TAIL_OP = None
LAST_RESULT = None


def _tail_ref(in0, in1, s0, s1, imm2):
    z = (in0.astype(np.float32) + s0) * (in0.astype(np.float32) + in1 + s1)
    return np.maximum(z, z * imm2).astype(np.float32)


def _get_tail_op():
    """Register the fused tail op with the ant custom-DVE table:
    out = max(Z, imm2*Z), Z = (in0 + s0)*(in0 + in1 + s1)."""
    global _TAIL_OP
    if _TAIL_OP is not None:
        return _TAIL_OP
    from concourse import dve_ops
    from concourse.dve_spec import C0, C1, C2, Spec, Src0, Src1, lower, maxx
    from concourse.dve_uop import DveOpSpec

    name = "CROSS_HEAD_TAIL_ANT"
    for op in dve_ops.OPS:
        if op.name == name:
            _TAIL_OP = op
            return op

    Z = (Src0 + C0) * (Src0 + Src1 + C1)
    spec = Spec(body=maxx(Z, Z * C2), reference=_tail_ref)
    row = max(dve_ops._SUB_OPCODE_FOR_NAME.values()) + 1
    assert row < 0x20
    shas = {}
    for ver in ("v3", "v4"):
        try:
            uops = lower(spec, ver=ver)
        except Exception:
            continue
        shas[ver] = DveOpSpec(
            name=name, opcode=row, uops=uops, rd1_en=True
        ).sha(ver)
    op = dve_ops.DveOp(name, spec, subdim=False, uops_sha=shas)
    dve_ops.OPS.append(op)
    dve_ops._SUB_OPCODE_FOR_NAME[name] = row
    dve_ops.CUSTOM_DVE_SPECS[name] = spec
    _TAIL_OP = op
    return op


def _build_nc(att_dtype="float8e4", amul_engine="gpsimd", sign_engine="scalar",
              smalls_engine="gpsimd", st_dtype="float8e4"):
    from contextlib import ExitStack

    import concourse.bacc as bacc
    import concourse.tile as tile
    from concourse import mybir

    tail_op = _get_tail_op()

    f32 = mybir.dt.float32
    bf16 = mybir.dt.bfloat16
    AL = mybir.AluOpType
    AF = mybir.ActivationFunctionType
    adt = getattr(mybir.dt, att_dtype)
    sdt = getattr(mybir.dt, st_dtype)

    nc = bacc.Bacc(
        "TRN2",
        target_bir_lowering=False,
        debug=False,
        enable_asserts=False,
        num_devices=NCORES,
    )
    x_d = nc.dram_tensor("x", [C, NT, PS * HWT], bf16, kind="ExternalInput").ap()
    a_d = nc.dram_tensor("att", [C, NT, FT], adt, kind="ExternalInput").ap()
    wt_d = nc.dram_tensor("wt", [C, C], bf16, kind="ExternalInput").ap()
    b_d = nc.dram_tensor("bias", [C, 2], f32, kind="ExternalInput").ap()
    o_d = nc.dram_tensor("out", [C, NT, PS * HWT], bf16, kind="ExternalOutput").ap()

    with tile.TileContext(nc) as tc, ExitStack() as ctx:
        const = ctx.enter_context(tc.tile_pool(name="const", bufs=1))
        wt_sb = const.tile([C, C], bf16)
        nc.sync.dma_start(wt_sb[:], wt_d[:])
        b_sb = const.tile([C, 2], f32)
        nc.sync.dma_start(b_sb[:], b_d[:])
        b_ap = b_sb[:, 0:1]
        bp1_ap = b_sb[:, 1:2]
        half_sb = const.tile([C, 1], f32)
        nc.vector.memset(half_sb[:], 0.5)

        xp = ctx.enter_context(tc.tile_pool(name="xp", bufs=4))
        atp = ctx.enter_context(tc.tile_pool(name="atp", bufs=4))
        sgp = ctx.enter_context(tc.tile_pool(name="sgp", bufs=3))
        r1p = ctx.enter_context(tc.tile_pool(name="r1p", bufs=3))
        nzp = ctx.enter_context(tc.tile_pool(name="nzp", bufs=3))
        Apl = ctx.enter_context(tc.tile_pool(name="Apl", bufs=3))
        ovp = ctx.enter_context(tc.tile_pool(name="ovp", bufs=4))
        psp = ctx.enter_context(tc.tile_pool(name="psp", bufs=2, space="PSUM"))

        def issue_loads(sub):
            xt = xp.tile([C, PS * HWT], bf16, name=f"xt{sub}", tag="xt")
            nc.sync.dma_start(xt[:], x_d[:, sub, :])
            at = atp.tile([C, FT], adt, name=f"at{sub}", tag="at")
            nc.sync.dma_start(at[:], a_d[:, sub, :])
            return xt, at

        loaded = {0: issue_loads(0), 1: issue_loads(1)}

        for sub in range(NT):
            xt, at = loaded.pop(sub)
            if sub + 2 < NT:
                loaded[sub + 2] = issue_loads(sub + 2)

            # ---- nz = count_nonzero per patch ----
            st = sgp.tile([C, FT], sdt)
            if sign_engine == "scalar":
                nc.scalar.activation(st[:], at[:], AF.Sign)
            else:
                nc.vector.tensor_scalar(st[:], at[:], 0.0, None, AL.not_equal)
            r1 = r1p.tile([C, PS * IWT], bf16)  # (p1, iw) counts over p2
            with nc.allow_low_precision(reason="counts <= 9 exact in bf16"):
                nc.vector.tensor_reduce(
                    r1[:],
                    st[:].rearrange("c (pw q) -> c pw q", q=PS),
                    mybir.AxisListType.X,
                    AL.add,
                    apply_absolute_value=True,
                )
            smalls = nc.gpsimd if smalls_engine == "gpsimd" else nc.vector
            nzv = nzp.tile([C, IWT], f32)
            nc.vector.tensor_reduce(
                nzv[:],
                r1[:].rearrange("c (p w) -> c w p", p=PS),
                mybir.AxisListType.X,
                AL.add,
            )
            # clamp to >=0.5 instead of +1e-5: identical output (for nz>=1
            # the 1e-5 shift is ~1e-7 relative; for nz=0 every att in the
            # patch is 0 so A = 0 either way), and it keeps 1/nz finite.
            nzc = nzp.tile([C, IWT], f32)
            nc.vector.tensor_scalar_max(nzc[:], nzv[:], 0.5)
            nzr = nzp.tile([C, IWT], f32)
            nc.vector.reciprocal_approx_fast(nzr[:], nzc[:])

            # ---- A = att * (1/nz); att already voxel order, contiguous --
            At = Apl.tile([C, FT], bf16)
            a3 = at[:].rearrange("c (p w q) -> c p w q", p=PS, q=PS)
            nzr3 = (
                nzr[:]
                .unsqueeze(1)
                .unsqueeze(3)
                .broadcast_to((C, PS, IWT, PS))
            )
            A3 = At[:].rearrange("c (p w q) -> c p w q", p=PS, q=PS)
            if amul_engine == "split":
                amul = nc.gpsimd if sub % 2 == 0 else nc.vector
            else:
                amul = getattr(nc, amul_engine)
            amul.tensor_tensor(A3, a3, nzr3, AL.mult)

            # ---- GEMM: psum = W @ x, plain contiguous voxel slices ----
            pst = []
            for g in range(NGRP):
                ps_t = psp.tile([C, MMG * BANK], f32)  # 3 banks
                pst.append(ps_t)
                for m in range(MMG):
                    ch = g * MMG + m
                    nc.tensor.matmul(
                        ps_t[:, m * BANK : m * BANK + MMN],
                        wt_sb[:],
                        xt[:, ch * MMN : (ch + 1) * MMN],
                        start=True,
                        stop=True,
                    )

            # ---- fused tail: out = lrelu((y0+b)*(y0+A+b+1), 0.2) ----
            ov = ovp.tile([C, PS * HWT], bf16)
            for g in range(NGRP):
                ps3 = (
                    pst[g][:]
                    .rearrange("c (m n) -> c m n", n=BANK)[:, :, 0:MMN]
                )  # [C, 3, 486]
                sl = slice(g * MMG * MMN, (g + 1) * MMG * MMN)
                nc.vector._custom_dve(
                    tail_op,
                    out=ov[:, sl],
                    in0=ps3,
                    in1=At[:, sl],
                    s0=b_ap,
                    s1=bp1_ap,
                    imm2=0.2,
                )

            # ---- contiguous store on the scalar HWDGE queue ----
            nc.scalar.dma_start(o_d[:, sub, :], ov[:])

    nc.compile()
    return nc


def _get_nc(**kw):
    key = tuple(sorted(kw.items()))
    if key not in _NC_CACHE:
        _NC_CACHE[key] = _build_nc(**kw)
    return _NC_CACHE[key]


def kernel(x, attentions, W, b, **build_kw):
    global LAST_RESULT
    import ml_dtypes
    from concourse.bass_utils import run_bass_kernel_spmd

    bf16 = ml_dtypes.bfloat16
    att_np = {"bfloat16": bf16, "float8e4": ml_dtypes.float8_e4m3}[
        build_kw.get("att_dtype", "float8e4")
    ]

    x = np.asarray(x, dtype=np.float32)
    attentions = np.asarray(attentions, dtype=np.float32)
    W = np.asarray(W, dtype=np.float32)
    b = np.asarray(b, dtype=np.float32)

    nc = _get_nc(**build_kw)

    # x: [C, NDP, PS, NCORES, NSUB, HWT] -> core-major, subtile-contiguous
    xs = (
        x.reshape(C, NDP, PS, NCORES, NSUB, HWT)
        .transpose(3, 0, 1, 4, 2, 5)
        .reshape(NCORES, C, NT, PS * HWT)
        .astype(bf16)
    )
    # att -> voxel order (p1, iw, p2) per subtile, core-major
    as_ = (
        attentions.reshape(C, NDP, NCORES, NSUB, IWT, PS, PS)
        .transpose(2, 0, 1, 3, 5, 4, 6)
        .reshape(NCORES, C, NT, FT)
        .astype(att_np)
    )
    wt = np.ascontiguousarray(W.T).astype(bf16)
    bcol = np.ascontiguousarray(np.stack([b, b + 1.0], axis=1))

    in_maps = []
    for s in range(NCORES):
        in_maps.append(
            {
                "x": np.ascontiguousarray(xs[s]),
                "att": np.ascontiguousarray(as_[s]),
                "wt": wt,
                "bias": bcol,
            }
        )

    res = run_bass_kernel_spmd(
        nc,
        in_maps,
        core_ids=list(range(NCORES)),
        trace=bool(os.environ.get("BASS_TRACE")),
    )
    LAST_RESULT = res

    out = np.empty((NCORES, C, NT, PS * HWT), dtype=np.float32)
    for s in range(NCORES):
        out[s] = res.results[s]["out"].astype(np.float32)
    # inverse of the x permutation
    full = (
        out.reshape(NCORES, C, NDP, NSUB, PS, HWT)
        .transpose(1, 2, 4, 0, 3, 5)
        .reshape(1, C, D, HWFULL)
    )
    return full
